# revision 34
# baseline (speedup 1.0000x reference)
"""Trainium2 Bass kernel for the Bayesian logistic-regression activation matrix.

Computes, for x [N, D], w_mu [D], w_log_var [D], z [NS]:
    mean  = x @ w_mu                       [N]
    var   = (x*x) @ exp(w_log_var)         [N]
    out[i, j] = sqrt(var_i) * z_j + mean_i [N, NS]

Data-parallel over 8 NeuronCores: rows of x sharded, everything else
replicated. Per core: 12500 rows = 25 tiles of R=500, grouped into
6 strips of 4 tiles + 1 strip of 1 tile.

Design (v3, tuned against a real HW trace):
  - x is pre-transposed AND pre-cast to bf16 on the host: DRAM tensor
    xt [4 chunks, 128 d, 12500 n]. Per-tile loads [128, 4, 500] have
    1000 B contiguous DRAM runs -> full 360 GB/s DMA rate. bf16 halves
    HBM read traffic vs f32.
  - squares on DVE (4x mode, one tensor_tensor per tile);
    exp(w_log_var) is folded into the var matmul's stationary vector.
  - PE does ONLY the two d-reductions per tile (4 chunk matmuls each,
    K=128, PSUM-accum): mean = wb_c^T @ x_c, var = eb_c^T @ sq_c.
    Measured 253 ns per [1,500] matmul (full clock) -> ~50 us/core.
    The v2 design also built the NS-broadcast on PE; its 200 extra
    K=1 matmuls cost 28 us in ldweights+stream, so v3 moves that to
    DVE (below) and PE runs gap-free on reductions alone.
  - std/mean rows (ACT sqrt / ACT copy out of PSUM) are written to
    partition 32*i of a strip row tile (engine SBUF writes must start
    at 32-aligned partitions), round-tripped through a DRAM scratch
    buffer, and reloaded as COLUMNS [125, 2*tps*4]. Only DRAM APs can
    re-partition data; SBUF->SBUF DMAs cannot scatter one partition.
  - output tile = DVE tensor_scalar: out[p, j] = z_j * std_p + mean_p
    (per-partition f32 scalars from the column tile, z replicated
    across partitions as a const). Writes the bf16 store tile
    directly - no PSUM bank, no eviction pass.
  - store: partition p holds output rows 4p..4p+3 of each tile
    contiguously -> 1024 B DRAM runs, one 3D DMA per strip.
  - DMA queue discipline: x loads on SP; the rows->scratch store on
    ACT (which wrote the rows, so it never waits); the cols reload
    and output store on GpSimd (otherwise idle). A dependent DMA
    parked on SP's queue stalls every later load (the sim showed a
    30 us PE starve from exactly that).
  - out is stored bf16 and upcast to f32 on host (halves write
    traffic; end-to-end max rel err ~5.6e-3 vs the 2e-2 gate).
"""

import numpy as np

N = 100000
D = 512
NS = 128
NCORES = 8
NSHARD = N // NCORES  # 12500 rows per core
P = 128               # SBUF partitions
C = D // P            # 4 chunks of the feature dim
R = 500               # rows per compute tile; PSUM row [1, 500] f32 = one bank
G = 4                 # output rows grouped per partition in the store
M = R // G            # 125 partitions used by the broadcast/store
TPS = 4               # tiles per strip (strip = 2000 rows; last strip 1 tile)

_CACHE = {}


def _build_bass(nshard=NSHARD):
    from contextlib import ExitStack

    import concourse.bacc as bacc
    import concourse.mybir as mybir
    import concourse.tile as tile
    from concourse.mybir import ActivationFunctionType as AFT

    f32 = mybir.dt.float32
    bf16 = mybir.dt.bfloat16
    AluOp = mybir.AluOpType

    ntiles = nshard // R
    # remainder strip FIRST: its short roundtrip chain hides in pipeline
    # fill, and the drain tail is identical for any trailing strip
    rem = ntiles % TPS
    strips = ([(0, rem)] if rem else []) + [
        (rem + k * TPS, TPS) for k in range((ntiles - rem) // TPS)
    ]
    nstrips = len(strips)

    nc = bacc.Bacc("TRN2", target_bir_lowering=False, debug=False)

    xt = nc.dram_tensor("xt", [C, P, nshard], bf16, kind="ExternalInput").ap()
    wb = nc.dram_tensor("wb", [P, C], bf16, kind="ExternalInput").ap()
    eb = nc.dram_tensor("eb", [P, C], bf16, kind="ExternalInput").ap()
    zrep = nc.dram_tensor("zrep", [P, NS], bf16, kind="ExternalInput").ap()
    out = nc.dram_tensor("out", [nshard, NS], bf16, kind="ExternalOutput").ap()
    # per-strip scratch for the row->column roundtrip; row 32*i of slice s
    # holds tile i's [std | mean] pair
    scr = nc.dram_tensor("scr", [nstrips, TPS, 2 * R], f32, kind="Internal").ap()

    with tile.TileContext(nc) as tc, ExitStack() as ctx:
        const_pool = ctx.enter_context(tc.tile_pool(name="const", bufs=1))
        xs_pool = ctx.enter_context(tc.tile_pool(name="xs", bufs=4))
        sq_pool = ctx.enter_context(tc.tile_pool(name="sq", bufs=4))
        rows_pool = ctx.enter_context(tc.tile_pool(name="rows", bufs=2))
        cols_pool = ctx.enter_context(tc.tile_pool(name="cols", bufs=3))
        osb_pool = ctx.enter_context(tc.tile_pool(name="osb", bufs=3))
        pm_pool = ctx.enter_context(tc.tile_pool(name="pm", bufs=4, space="PSUM"))
        pv_pool = ctx.enter_context(tc.tile_pool(name="pv", bufs=4, space="PSUM"))

        w_t = const_pool.tile([P, C], bf16)
        nc.sync.dma_start(w_t[:], wb[:])
        e_t = const_pool.tile([P, C], bf16)
        nc.sync.dma_start(e_t[:], eb[:])
        z_t = const_pool.tile([P, NS], bf16)
        nc.sync.dma_start(z_t[:], zrep[:])

        def start_broadcast(job):
            """Allocate the store tile for a finished strip's broadcast."""
            s0, tps, cols_t = job
            osb_t = osb_pool.tile([M, tps * G * NS], bf16)
            return (s0, tps, cols_t, osb_t)

        def emit_broadcast_chunk(bjob, i):
            """One tile's worth (G ops) of DVE broadcast for strip bjob.
            Interleaved between the current strip's tiles so the next
            squares op is never parked behind 16 queued broadcasts."""
            s0, tps, cols_t, osb_t = bjob
            if i >= tps:
                return
            for b in range(G):
                ib = i * G + b
                if ib % 3 == 2:
                    # every third broadcast on ACT (Identity table entry is
                    # co-resident with Sqrt/Copy, so no table reloads):
                    # out = Identity(z * std_p + mean_p)
                    nc.scalar.activation(
                        osb_t[:, ib * NS : (ib + 1) * NS],
                        z_t[0:M, :],
                        AFT.Identity,
                        bias=cols_t[:, tps * G + ib : tps * G + ib + 1],
                        scale=cols_t[:, ib : ib + 1],
                    )
                else:
                    nc.vector.tensor_scalar(
                        osb_t[:, ib * NS : (ib + 1) * NS],
                        z_t[0:M, :],
                        cols_t[:, ib : ib + 1],
                        cols_t[:, tps * G + ib : tps * G + ib + 1],
                        AluOp.mult,
                        AluOp.add,
                    )

        def emit_store(job):
            # strip store: one 3D DMA; partition p covers output rows
            # s0 + i*R + 4p .. +3 for each tile i (1KB contiguous runs).
            # Issued from SP (its queue spreads over 15 DMA engines; the
            # gpsimd/ACT queues all land on DMA engine 0) two strips after
            # the broadcast, so it never waits at the head of SP's queue.
            s0, tps, osb_t = job
            nc.sync.dma_start(
                out[s0 : s0 + tps * R, :].rearrange(
                    "(i p b) j -> p i (b j)", i=tps, b=G
                ),
                osb_t[:],
            )

        bjob = None        # strip s-1: broadcast interleaved through strip s
        pend_store = None  # strip s-2: store during strip s

        for s, (tbase, tps) in enumerate(strips):
            s0 = tbase * R
            S = tps * R

            xs_t = xs_pool.tile([P, C * S], bf16)
            sq_t = sq_pool.tile([P, C * S], bf16)
            # tile i's std at [32i, 0:R], mean at [32i, R:2R]
            rows_t = rows_pool.tile([32 * (tps - 1) + 1, 2 * R], f32)

            for i in range(tps):
                w0 = i * R
                nc.sync.dma_start(
                    xs_t[:].rearrange("p (c n) -> p c n", c=C)[:, :, w0 : w0 + R],
                    xt[:, :, s0 + w0 : s0 + w0 + R].rearrange("c p n -> p c n"),
                )

            # whole-strip squares: one FLAT 2D op (a 3D chunked AP runs at
            # DVE 2x on HW; flat stride-1 is the best shot at 4x mode)
            nc.vector.tensor_tensor(sq_t[:], xs_t[:], xs_t[:], AluOp.mult)

            for i in range(tps):
                w0 = i * R

                # d-reductions on PE, PSUM-accumulated over the 4 chunks
                pmean = pm_pool.tile([1, R], f32)
                for c in range(C):
                    nc.tensor.matmul(
                        pmean[:],
                        w_t[:, c : c + 1],
                        xs_t[:, c * S + w0 : c * S + w0 + R],
                        start=(c == 0),
                        stop=(c == C - 1),
                    )
                pvar = pv_pool.tile([1, R], f32)
                for c in range(C):
                    nc.tensor.matmul(
                        pvar[:],
                        e_t[:, c : c + 1],
                        sq_t[:, c * S + w0 : c * S + w0 + R],
                        start=(c == 0),
                        stop=(c == C - 1),
                    )

                # f32 std/mean rows on partition 32*i of the strip row tile
                pi = 32 * i
                nc.scalar.sqrt(rows_t[pi : pi + 1, 0:R], pvar[:])
                nc.scalar.copy(rows_t[pi : pi + 1, R : 2 * R], pmean[:])

                # one tile's worth of the previous strip's broadcast
                if bjob is not None:
                    emit_broadcast_chunk(bjob, i)

            # roundtrip: rows -> DRAM -> columns [125, 2*tps*G].
            # The scratch store is dispatched from ACT (which wrote the
            # rows, so it never waits at the head of the queue); the cols
            # reload waits on GpSimd, keeping SP free to stream x loads.
            npart = 32 * (tps - 1) + 1
            nc.scalar.dma_start(
                scr[s, 0:tps], rows_t[0 : npart : 32, :]
            )
            cols_t = cols_pool.tile([M, 2 * tps * G], f32)
            if tps == 1:
                nc.gpsimd.dma_start(
                    cols_t[:].rearrange("p (k b) -> p k b", k=2),
                    scr[s, 0].rearrange("(k p b) -> p k b", k=2, b=G),
                )
            else:
                scr_v = scr[s, 0:tps].rearrange("i (k p b) -> k p i b", k=2, b=G)
                for k in range(2):
                    nc.gpsimd.dma_start(
                        cols_t[:, k * tps * G : (k + 1) * tps * G].rearrange(
                            "p (i b) -> p i b", i=tps
                        ),
                        scr_v[k],
                    )

            # finish any broadcast chunks the interleave didn't cover
            # (previous strip had more tiles than this one)
            if bjob is not None:
                for i in range(tps, bjob[1]):
                    emit_broadcast_chunk(bjob, i)

            if pend_store is not None:
                emit_store(pend_store)
                pend_store = None
            if bjob is not None:
                pend_store = (bjob[0], bjob[1], bjob[3])
            bjob = start_broadcast((s0, tps, cols_t))

        if pend_store is not None:
            emit_store(pend_store)
        for i in range(bjob[1]):
            emit_broadcast_chunk(bjob, i)
        emit_store((bjob[0], bjob[1], bjob[3]))

    nc.compile()
    return nc


def _host_consts(w_mu, w_log_var, z):
    import ml_dtypes

    bf16 = ml_dtypes.bfloat16
    e = np.exp(w_log_var.astype(np.float64))
    wb = np.ascontiguousarray(w_mu.reshape(C, P).T).astype(bf16)
    eb = np.ascontiguousarray(e.reshape(C, P).T).astype(bf16)
    zrep = np.ascontiguousarray(
        np.broadcast_to(z.astype(np.float32), (P, NS))
    ).astype(bf16)
    return wb, eb, zrep


def _get_nc():
    if "nc" not in _CACHE:
        _CACHE["nc"] = _build_bass()
    return _CACHE["nc"]


def kernel(x, w_mu, w_log_var, z, _trace=False, _tmpdir=None):
    import ml_dtypes

    from concourse.bass_utils import run_bass_kernel_spmd

    bf16 = ml_dtypes.bfloat16

    x = np.ascontiguousarray(x, dtype=np.float32)
    w_mu = np.asarray(w_mu, dtype=np.float32)
    w_log_var = np.asarray(w_log_var, dtype=np.float32)
    z = np.asarray(z, dtype=np.float32)

    # [8, 512, 12500] chunk-major transposed bf16 copy of x
    xtb = x.reshape(NCORES, NSHARD, D).transpose(0, 2, 1).astype(bf16)

    wb, eb, zrep = _host_consts(w_mu, w_log_var, z)

    in_maps = []
    for c in range(NCORES):
        in_maps.append(
            {
                "xt": xtb[c].reshape(C, P, NSHARD),
                "wb": wb,
                "eb": eb,
                "zrep": zrep,
            }
        )

    nc = _get_nc()
    res = run_bass_kernel_spmd(
        nc,
        in_maps,
        core_ids=list(range(NCORES)),
        trace=_trace,
        tmpdir=_tmpdir,
        stitch_traces=False,
    )
    _CACHE["last_results"] = res
    outs = [np.asarray(r["out"]).astype(np.float32) for r in res.results]
    return np.concatenate(outs, axis=0)


# revision 35
# speedup vs baseline: 1.0012x; 1.0012x over previous
"""Trainium2 Bass kernel for the Bayesian logistic-regression activation matrix.

Computes, for x [N, D], w_mu [D], w_log_var [D], z [NS]:
    mean  = x @ w_mu                       [N]
    var   = (x*x) @ exp(w_log_var)         [N]
    out[i, j] = sqrt(var_i) * z_j + mean_i [N, NS]

Data-parallel over 8 NeuronCores: rows of x sharded, everything else
replicated. Per core: 12500 rows = 25 tiles of R=500, grouped into
6 strips of 4 tiles + 1 strip of 1 tile.

Design (v3, tuned against a real HW trace):
  - x is pre-transposed AND pre-cast to bf16 on the host: DRAM tensor
    xt [4 chunks, 128 d, 12500 n]. Per-tile loads [128, 4, 500] have
    1000 B contiguous DRAM runs -> full 360 GB/s DMA rate. bf16 halves
    HBM read traffic vs f32.
  - squares on DVE (4x mode, one tensor_tensor per tile);
    exp(w_log_var) is folded into the var matmul's stationary vector.
  - PE does ONLY the two d-reductions per tile (4 chunk matmuls each,
    K=128, PSUM-accum): mean = wb_c^T @ x_c, var = eb_c^T @ sq_c.
    Measured 253 ns per [1,500] matmul (full clock) -> ~50 us/core.
    The v2 design also built the NS-broadcast on PE; its 200 extra
    K=1 matmuls cost 28 us in ldweights+stream, so v3 moves that to
    DVE (below) and PE runs gap-free on reductions alone.
  - std/mean rows (ACT sqrt / ACT copy out of PSUM) are written to
    partition 32*i of a strip row tile (engine SBUF writes must start
    at 32-aligned partitions), round-tripped through a DRAM scratch
    buffer, and reloaded as COLUMNS [125, 2*tps*4]. Only DRAM APs can
    re-partition data; SBUF->SBUF DMAs cannot scatter one partition.
  - output tile = DVE tensor_scalar: out[p, j] = z_j * std_p + mean_p
    (per-partition f32 scalars from the column tile, z replicated
    across partitions as a const). Writes the bf16 store tile
    directly - no PSUM bank, no eviction pass.
  - store: partition p holds output rows 4p..4p+3 of each tile
    contiguously -> 1024 B DRAM runs, one 3D DMA per strip.
  - DMA queue discipline: x loads on SP; the rows->scratch store on
    ACT (which wrote the rows, so it never waits); the cols reload
    and output store on GpSimd (otherwise idle). A dependent DMA
    parked on SP's queue stalls every later load (the sim showed a
    30 us PE starve from exactly that).
  - out is stored bf16 and upcast to f32 on host (halves write
    traffic; end-to-end max rel err ~5.6e-3 vs the 2e-2 gate).
"""

import numpy as np

N = 100000
D = 512
NS = 128
NCORES = 8
NSHARD = N // NCORES  # 12500 rows per core
P = 128               # SBUF partitions
C = D // P            # 4 chunks of the feature dim
R = 500               # rows per compute tile; PSUM row [1, 500] f32 = one bank
G = 4                 # output rows grouped per partition in the store
M = R // G            # 125 partitions used by the broadcast/store
TPS = 4               # tiles per strip (strip = 2000 rows; last strip 1 tile)

_CACHE = {}


def _build_bass(nshard=NSHARD):
    from contextlib import ExitStack

    import concourse.bacc as bacc
    import concourse.mybir as mybir
    import concourse.tile as tile
    from concourse.mybir import ActivationFunctionType as AFT

    f32 = mybir.dt.float32
    bf16 = mybir.dt.bfloat16
    AluOp = mybir.AluOpType

    ntiles = nshard // R
    # remainder strip FIRST: its short roundtrip chain hides in pipeline
    # fill, and the drain tail is identical for any trailing strip
    rem = ntiles % TPS
    strips = ([(0, rem)] if rem else []) + [
        (rem + k * TPS, TPS) for k in range((ntiles - rem) // TPS)
    ]
    nstrips = len(strips)

    nc = bacc.Bacc("TRN2", target_bir_lowering=False, debug=False)

    xt = nc.dram_tensor("xt", [C, P, nshard], bf16, kind="ExternalInput").ap()
    wb = nc.dram_tensor("wb", [P, C], bf16, kind="ExternalInput").ap()
    eb = nc.dram_tensor("eb", [P, C], bf16, kind="ExternalInput").ap()
    zrep = nc.dram_tensor("zrep", [P, NS], bf16, kind="ExternalInput").ap()
    out = nc.dram_tensor("out", [nshard, NS], bf16, kind="ExternalOutput").ap()
    # per-strip scratch for the row->column roundtrip; row 32*i of slice s
    # holds tile i's [std | mean] pair
    scr = nc.dram_tensor("scr", [nstrips, TPS, 2 * R], f32, kind="Internal").ap()

    with tile.TileContext(nc) as tc, ExitStack() as ctx:
        const_pool = ctx.enter_context(tc.tile_pool(name="const", bufs=1))
        xs_pool = ctx.enter_context(tc.tile_pool(name="xs", bufs=4))
        sq_pool = ctx.enter_context(tc.tile_pool(name="sq", bufs=4))
        rows_pool = ctx.enter_context(tc.tile_pool(name="rows", bufs=2))
        cols_pool = ctx.enter_context(tc.tile_pool(name="cols", bufs=3))
        osb_pool = ctx.enter_context(tc.tile_pool(name="osb", bufs=3))
        pm_pool = ctx.enter_context(tc.tile_pool(name="pm", bufs=4, space="PSUM"))
        pv_pool = ctx.enter_context(tc.tile_pool(name="pv", bufs=4, space="PSUM"))

        w_t = const_pool.tile([P, C], bf16)
        nc.sync.dma_start(w_t[:], wb[:])
        e_t = const_pool.tile([P, C], bf16)
        nc.sync.dma_start(e_t[:], eb[:])
        z_t = const_pool.tile([P, NS], bf16)
        nc.sync.dma_start(z_t[:], zrep[:])

        def start_broadcast(job):
            """Allocate the store tile for a finished strip's broadcast."""
            s0, tps, cols_t = job
            osb_t = osb_pool.tile([M, tps * G * NS], bf16)
            return (s0, tps, cols_t, osb_t)

        def emit_broadcast_chunk(bjob, i):
            """One tile's worth (G ops) of DVE broadcast for strip bjob.
            Interleaved between the current strip's tiles so the next
            squares op is never parked behind 16 queued broadcasts."""
            s0, tps, cols_t, osb_t = bjob
            if i >= tps:
                return
            for b in range(G):
                ib = i * G + b
                nc.vector.tensor_scalar(
                    osb_t[:, ib * NS : (ib + 1) * NS],
                    z_t[0:M, :],
                    cols_t[:, ib : ib + 1],
                    cols_t[:, tps * G + ib : tps * G + ib + 1],
                    AluOp.mult,
                    AluOp.add,
                )

        def emit_store(job):
            # strip store: one 3D DMA; partition p covers output rows
            # s0 + i*R + 4p .. +3 for each tile i (1KB contiguous runs).
            # Issued from SP (its queue spreads over 15 DMA engines; the
            # gpsimd/ACT queues all land on DMA engine 0) two strips after
            # the broadcast, so it never waits at the head of SP's queue.
            s0, tps, osb_t = job
            nc.sync.dma_start(
                out[s0 : s0 + tps * R, :].rearrange(
                    "(i p b) j -> p i (b j)", i=tps, b=G
                ),
                osb_t[:],
            )

        bjob = None        # strip s-1: broadcast interleaved through strip s
        pend_store = None  # strip s-2: store during strip s

        for s, (tbase, tps) in enumerate(strips):
            s0 = tbase * R
            S = tps * R

            xs_t = xs_pool.tile([P, C * S], bf16)
            sq_t = sq_pool.tile([P, C * S], bf16)
            # tile i's std at [32i, 0:R], mean at [32i, R:2R]
            rows_t = rows_pool.tile([32 * (tps - 1) + 1, 2 * R], f32)

            for i in range(tps):
                w0 = i * R
                nc.sync.dma_start(
                    xs_t[:].rearrange("p (c n) -> p c n", c=C)[:, :, w0 : w0 + R],
                    xt[:, :, s0 + w0 : s0 + w0 + R].rearrange("c p n -> p c n"),
                )

            # whole-strip squares: one FLAT 2D op (a 3D chunked AP runs at
            # DVE 2x on HW; flat stride-1 is the best shot at 4x mode)
            nc.vector.tensor_tensor(sq_t[:], xs_t[:], xs_t[:], AluOp.mult)

            for i in range(tps):
                w0 = i * R

                # d-reductions on PE, PSUM-accumulated over the 4 chunks
                pmean = pm_pool.tile([1, R], f32)
                for c in range(C):
                    nc.tensor.matmul(
                        pmean[:],
                        w_t[:, c : c + 1],
                        xs_t[:, c * S + w0 : c * S + w0 + R],
                        start=(c == 0),
                        stop=(c == C - 1),
                    )
                pvar = pv_pool.tile([1, R], f32)
                for c in range(C):
                    nc.tensor.matmul(
                        pvar[:],
                        e_t[:, c : c + 1],
                        sq_t[:, c * S + w0 : c * S + w0 + R],
                        start=(c == 0),
                        stop=(c == C - 1),
                    )

                # f32 std/mean rows on partition 32*i of the strip row tile
                pi = 32 * i
                nc.scalar.sqrt(rows_t[pi : pi + 1, 0:R], pvar[:])
                nc.scalar.copy(rows_t[pi : pi + 1, R : 2 * R], pmean[:])

                # one tile's worth of the previous strip's broadcast
                if bjob is not None:
                    emit_broadcast_chunk(bjob, i)

            # roundtrip: rows -> DRAM -> columns [125, 2*tps*G].
            # The scratch store is dispatched from ACT (which wrote the
            # rows, so it never waits at the head of the queue); the cols
            # reload waits on GpSimd, keeping SP free to stream x loads.
            npart = 32 * (tps - 1) + 1
            nc.scalar.dma_start(
                scr[s, 0:tps], rows_t[0 : npart : 32, :]
            )
            cols_t = cols_pool.tile([M, 2 * tps * G], f32)
            if tps == 1:
                nc.gpsimd.dma_start(
                    cols_t[:].rearrange("p (k b) -> p k b", k=2),
                    scr[s, 0].rearrange("(k p b) -> p k b", k=2, b=G),
                )
            else:
                scr_v = scr[s, 0:tps].rearrange("i (k p b) -> k p i b", k=2, b=G)
                for k in range(2):
                    nc.gpsimd.dma_start(
                        cols_t[:, k * tps * G : (k + 1) * tps * G].rearrange(
                            "p (i b) -> p i b", i=tps
                        ),
                        scr_v[k],
                    )

            # finish any broadcast chunks the interleave didn't cover
            # (previous strip had more tiles than this one)
            if bjob is not None:
                for i in range(tps, bjob[1]):
                    emit_broadcast_chunk(bjob, i)

            if pend_store is not None:
                emit_store(pend_store)
                pend_store = None
            if bjob is not None:
                pend_store = (bjob[0], bjob[1], bjob[3])
            bjob = start_broadcast((s0, tps, cols_t))

        if pend_store is not None:
            emit_store(pend_store)
        for i in range(bjob[1]):
            emit_broadcast_chunk(bjob, i)
        emit_store((bjob[0], bjob[1], bjob[3]))

    nc.compile()
    return nc


def _host_consts(w_mu, w_log_var, z):
    import ml_dtypes

    bf16 = ml_dtypes.bfloat16
    e = np.exp(w_log_var.astype(np.float64))
    wb = np.ascontiguousarray(w_mu.reshape(C, P).T).astype(bf16)
    eb = np.ascontiguousarray(e.reshape(C, P).T).astype(bf16)
    zrep = np.ascontiguousarray(
        np.broadcast_to(z.astype(np.float32), (P, NS))
    ).astype(bf16)
    return wb, eb, zrep


def _get_nc():
    if "nc" not in _CACHE:
        _CACHE["nc"] = _build_bass()
    return _CACHE["nc"]


def kernel(x, w_mu, w_log_var, z, _trace=False, _tmpdir=None):
    import ml_dtypes

    from concourse.bass_utils import run_bass_kernel_spmd

    bf16 = ml_dtypes.bfloat16

    x = np.ascontiguousarray(x, dtype=np.float32)
    w_mu = np.asarray(w_mu, dtype=np.float32)
    w_log_var = np.asarray(w_log_var, dtype=np.float32)
    z = np.asarray(z, dtype=np.float32)

    # [8, 512, 12500] chunk-major transposed bf16 copy of x
    xtb = x.reshape(NCORES, NSHARD, D).transpose(0, 2, 1).astype(bf16)

    wb, eb, zrep = _host_consts(w_mu, w_log_var, z)

    in_maps = []
    for c in range(NCORES):
        in_maps.append(
            {
                "xt": xtb[c].reshape(C, P, NSHARD),
                "wb": wb,
                "eb": eb,
                "zrep": zrep,
            }
        )

    nc = _get_nc()
    res = run_bass_kernel_spmd(
        nc,
        in_maps,
        core_ids=list(range(NCORES)),
        trace=_trace,
        tmpdir=_tmpdir,
        stitch_traces=False,
    )
    _CACHE["last_results"] = res
    outs = [np.asarray(r["out"]).astype(np.float32) for r in res.results]
    return np.concatenate(outs, axis=0)


# revision 36
# speedup vs baseline: 1.0804x; 1.0792x over previous
"""Trainium2 Bass kernel for the Bayesian logistic-regression activation matrix.

Computes, for x [N, D], w_mu [D], w_log_var [D], z [NS]:
    mean  = x @ w_mu                       [N]
    var   = (x*x) @ exp(w_log_var)         [N]
    out[i, j] = sqrt(var_i) * z_j + mean_i [N, NS]

Data-parallel over 8 NeuronCores: rows of x sharded, everything else
replicated. Per core: 12500 rows = 25 tiles of R=500, grouped into
6 strips of 4 tiles + 1 strip of 1 tile.

Design (v3, tuned against a real HW trace):
  - x is pre-transposed AND pre-cast to bf16 on the host: DRAM tensor
    xt [4 chunks, 128 d, 12500 n]. Per-tile loads [128, 4, 500] have
    1000 B contiguous DRAM runs -> full 360 GB/s DMA rate. bf16 halves
    HBM read traffic vs f32.
  - squares on DVE (4x mode, one tensor_tensor per tile);
    exp(w_log_var) is folded into the var matmul's stationary vector.
  - PE does ONLY the two d-reductions per tile (4 chunk matmuls each,
    K=128, PSUM-accum): mean = wb_c^T @ x_c, var = eb_c^T @ sq_c.
    Measured 253 ns per [1,500] matmul (full clock) -> ~50 us/core.
    The v2 design also built the NS-broadcast on PE; its 200 extra
    K=1 matmuls cost 28 us in ldweights+stream, so v3 moves that to
    DVE (below) and PE runs gap-free on reductions alone.
  - std/mean rows (ACT sqrt / ACT copy out of PSUM) are written to
    partition 32*i of a strip row tile (engine SBUF writes must start
    at 32-aligned partitions), round-tripped through a DRAM scratch
    buffer, and reloaded as COLUMNS [125, 2*tps*4]. Only DRAM APs can
    re-partition data; SBUF->SBUF DMAs cannot scatter one partition.
  - output tile = DVE tensor_scalar: out[p, j] = z_j * std_p + mean_p
    (per-partition f32 scalars from the column tile, z replicated
    across partitions as a const). Writes the bf16 store tile
    directly - no PSUM bank, no eviction pass.
  - store: partition p holds output rows 4p..4p+3 of each tile
    contiguously -> 1024 B DRAM runs, one 3D DMA per strip.
  - DMA queue discipline: x loads on SP; the rows->scratch store on
    ACT (which wrote the rows, so it never waits); the cols reload
    and output store on GpSimd (otherwise idle). A dependent DMA
    parked on SP's queue stalls every later load (the sim showed a
    30 us PE starve from exactly that).
  - out is stored bf16 and upcast to f32 on host (halves write
    traffic; end-to-end max rel err ~5.6e-3 vs the 2e-2 gate).
"""

import numpy as np

N = 100000
D = 512
NS = 128
NCORES = 8
NSHARD = N // NCORES  # 12500 rows per core
P = 128               # SBUF partitions
C = D // P            # 4 chunks of the feature dim
R = 500               # rows per compute tile; PSUM row [1, 500] f32 = one bank
G = 4                 # output rows grouped per partition in the store
M = R // G            # 125 partitions used by the broadcast/store
TPS = 4               # tiles per strip (strip = 2000 rows; last strip 1 tile)

_CACHE = {}


def _build_bass(nshard=NSHARD):
    from contextlib import ExitStack

    import concourse.bacc as bacc
    import concourse.mybir as mybir
    import concourse.tile as tile
    from concourse.mybir import ActivationFunctionType as AFT

    f32 = mybir.dt.float32
    bf16 = mybir.dt.bfloat16
    AluOp = mybir.AluOpType

    ntiles = nshard // R
    # remainder strip LAST: the drain tail scales with the final strip's
    # roundtrip chain, so the 1-tile strip belongs at the end
    strips = []
    t0 = 0
    while t0 < ntiles:
        tps = min(TPS, ntiles - t0)
        strips.append((t0, tps))
        t0 += tps
    nstrips = len(strips)

    nc = bacc.Bacc("TRN2", target_bir_lowering=False, debug=False)

    xt = nc.dram_tensor("xt", [C, P, nshard], bf16, kind="ExternalInput").ap()
    wb = nc.dram_tensor("wb", [P, C], bf16, kind="ExternalInput").ap()
    eb = nc.dram_tensor("eb", [P, C], bf16, kind="ExternalInput").ap()
    zrep = nc.dram_tensor("zrep", [P, NS], bf16, kind="ExternalInput").ap()
    out = nc.dram_tensor("out", [nshard, NS], bf16, kind="ExternalOutput").ap()
    # per-strip scratch for the row->column roundtrip; row 32*i of slice s
    # holds tile i's [std | mean] pair
    scr = nc.dram_tensor("scr", [nstrips, TPS, 2 * R], f32, kind="Internal").ap()

    with tile.TileContext(nc) as tc, ExitStack() as ctx:
        const_pool = ctx.enter_context(tc.tile_pool(name="const", bufs=1))
        xs_pool = ctx.enter_context(tc.tile_pool(name="xs", bufs=4))
        sq_pool = ctx.enter_context(tc.tile_pool(name="sq", bufs=4))
        rows_pool = ctx.enter_context(tc.tile_pool(name="rows", bufs=2))
        cols_pool = ctx.enter_context(tc.tile_pool(name="cols", bufs=3))
        osb_pool = ctx.enter_context(tc.tile_pool(name="osb", bufs=3))
        pm_pool = ctx.enter_context(tc.tile_pool(name="pm", bufs=4, space="PSUM"))
        pv_pool = ctx.enter_context(tc.tile_pool(name="pv", bufs=4, space="PSUM"))

        w_t = const_pool.tile([P, C], bf16)
        nc.sync.dma_start(w_t[:], wb[:])
        e_t = const_pool.tile([P, C], bf16)
        nc.sync.dma_start(e_t[:], eb[:])
        z_t = const_pool.tile([P, NS], bf16)
        nc.sync.dma_start(z_t[:], zrep[:])

        def start_broadcast(job):
            """Allocate the store tile for a finished strip's broadcast."""
            s0, tps, cols_t = job
            osb_t = osb_pool.tile([M, tps * G * NS], bf16)
            return (s0, tps, cols_t, osb_t)

        def emit_broadcast_chunk(bjob, i):
            """One tile's worth (G ops) of DVE broadcast for strip bjob.
            Interleaved between the current strip's tiles so the next
            squares op is never parked behind 16 queued broadcasts."""
            s0, tps, cols_t, osb_t = bjob
            if i >= tps:
                return
            for b in range(G):
                ib = i * G + b
                nc.vector.tensor_scalar(
                    osb_t[:, ib * NS : (ib + 1) * NS],
                    z_t[0:M, :],
                    cols_t[:, ib : ib + 1],
                    cols_t[:, tps * G + ib : tps * G + ib + 1],
                    AluOp.mult,
                    AluOp.add,
                )

        def emit_store(job):
            # strip store: one 3D DMA; partition p covers output rows
            # s0 + i*R + 4p .. +3 for each tile i (1KB contiguous runs).
            # Issued from SP (its queue spreads over 15 DMA engines; the
            # gpsimd/ACT queues all land on DMA engine 0) two strips after
            # the broadcast, so it never waits at the head of SP's queue.
            s0, tps, osb_t = job
            nc.sync.dma_start(
                out[s0 : s0 + tps * R, :].rearrange(
                    "(i p b) j -> p i (b j)", i=tps, b=G
                ),
                osb_t[:],
            )

        bjob = None        # strip s-1: broadcast interleaved through strip s
        pend_store = None  # strip s-2: store during strip s

        for s, (tbase, tps) in enumerate(strips):
            s0 = tbase * R
            S = tps * R

            xs_t = xs_pool.tile([P, C * S], bf16)
            sq_t = sq_pool.tile([P, C * S], bf16)
            # tile i's std at [32i, 0:R], mean at [32i, R:2R]
            rows_t = rows_pool.tile([32 * (tps - 1) + 1, 2 * R], f32)

            for i in range(tps):
                w0 = i * R
                nc.sync.dma_start(
                    xs_t[:].rearrange("p (c n) -> p c n", c=C)[:, :, w0 : w0 + R],
                    xt[:, :, s0 + w0 : s0 + w0 + R].rearrange("c p n -> p c n"),
                )

            # whole-strip squares: one FLAT 2D op (a 3D chunked AP runs at
            # DVE 2x on HW; flat stride-1 is the best shot at 4x mode)
            nc.vector.tensor_tensor(sq_t[:], xs_t[:], xs_t[:], AluOp.mult)

            for i in range(tps):
                w0 = i * R

                # d-reductions on PE, PSUM-accumulated over the 4 chunks
                pmean = pm_pool.tile([1, R], f32)
                for c in range(C):
                    nc.tensor.matmul(
                        pmean[:],
                        w_t[:, c : c + 1],
                        xs_t[:, c * S + w0 : c * S + w0 + R],
                        start=(c == 0),
                        stop=(c == C - 1),
                    )
                pvar = pv_pool.tile([1, R], f32)
                for c in range(C):
                    nc.tensor.matmul(
                        pvar[:],
                        e_t[:, c : c + 1],
                        sq_t[:, c * S + w0 : c * S + w0 + R],
                        start=(c == 0),
                        stop=(c == C - 1),
                    )

                # f32 std/mean rows on partition 32*i of the strip row tile
                pi = 32 * i
                nc.scalar.sqrt(rows_t[pi : pi + 1, 0:R], pvar[:])
                nc.scalar.copy(rows_t[pi : pi + 1, R : 2 * R], pmean[:])

                # one tile's worth of the previous strip's broadcast
                if bjob is not None:
                    emit_broadcast_chunk(bjob, i)

            # roundtrip: rows -> DRAM -> columns [125, 2*tps*G].
            # The scratch store is dispatched from ACT (which wrote the
            # rows, so it never waits at the head of the queue); the cols
            # reload waits on GpSimd, keeping SP free to stream x loads.
            npart = 32 * (tps - 1) + 1
            nc.scalar.dma_start(
                scr[s, 0:tps], rows_t[0 : npart : 32, :]
            )
            cols_t = cols_pool.tile([M, 2 * tps * G], f32)
            if tps == 1:
                nc.gpsimd.dma_start(
                    cols_t[:].rearrange("p (k b) -> p k b", k=2),
                    scr[s, 0].rearrange("(k p b) -> p k b", k=2, b=G),
                )
            else:
                scr_v = scr[s, 0:tps].rearrange("i (k p b) -> k p i b", k=2, b=G)
                for k in range(2):
                    nc.gpsimd.dma_start(
                        cols_t[:, k * tps * G : (k + 1) * tps * G].rearrange(
                            "p (i b) -> p i b", i=tps
                        ),
                        scr_v[k],
                    )

            # finish any broadcast chunks the interleave didn't cover
            # (previous strip had more tiles than this one)
            if bjob is not None:
                for i in range(tps, bjob[1]):
                    emit_broadcast_chunk(bjob, i)

            if pend_store is not None:
                emit_store(pend_store)
                pend_store = None
            if bjob is not None:
                pend_store = (bjob[0], bjob[1], bjob[3])
            bjob = start_broadcast((s0, tps, cols_t))

        if pend_store is not None:
            emit_store(pend_store)
        for i in range(bjob[1]):
            emit_broadcast_chunk(bjob, i)
        emit_store((bjob[0], bjob[1], bjob[3]))

    nc.compile()
    return nc


def _host_consts(w_mu, w_log_var, z):
    import ml_dtypes

    bf16 = ml_dtypes.bfloat16
    e = np.exp(w_log_var.astype(np.float64))
    wb = np.ascontiguousarray(w_mu.reshape(C, P).T).astype(bf16)
    eb = np.ascontiguousarray(e.reshape(C, P).T).astype(bf16)
    zrep = np.ascontiguousarray(
        np.broadcast_to(z.astype(np.float32), (P, NS))
    ).astype(bf16)
    return wb, eb, zrep


def _get_nc():
    if "nc" not in _CACHE:
        _CACHE["nc"] = _build_bass()
    return _CACHE["nc"]


def kernel(x, w_mu, w_log_var, z, _trace=False, _tmpdir=None):
    import ml_dtypes

    from concourse.bass_utils import run_bass_kernel_spmd

    bf16 = ml_dtypes.bfloat16

    x = np.ascontiguousarray(x, dtype=np.float32)
    w_mu = np.asarray(w_mu, dtype=np.float32)
    w_log_var = np.asarray(w_log_var, dtype=np.float32)
    z = np.asarray(z, dtype=np.float32)

    # [8, 512, 12500] chunk-major transposed bf16 copy of x
    xtb = x.reshape(NCORES, NSHARD, D).transpose(0, 2, 1).astype(bf16)

    wb, eb, zrep = _host_consts(w_mu, w_log_var, z)

    in_maps = []
    for c in range(NCORES):
        in_maps.append(
            {
                "xt": xtb[c].reshape(C, P, NSHARD),
                "wb": wb,
                "eb": eb,
                "zrep": zrep,
            }
        )

    nc = _get_nc()
    res = run_bass_kernel_spmd(
        nc,
        in_maps,
        core_ids=list(range(NCORES)),
        trace=_trace,
        tmpdir=_tmpdir,
        stitch_traces=False,
    )
    _CACHE["last_results"] = res
    outs = [np.asarray(r["out"]).astype(np.float32) for r in res.results]
    return np.concatenate(outs, axis=0)
